# revision 29
# baseline (speedup 1.0000x reference)
"""Trainium2 Bass kernel for nn_Attention_24215025615017.

8-head spatial attention block (1x1-conv QKV projections with folded BatchNorm,
transposed-softmax attention, exact GELU, output 1x1 conv with folded BN).
Data-parallel over batch: B=32 sharded as 4 batches on each of 8 NeuronCores.

Pipeline design: the softmax exp on the Scalar engine (~64 ACTs/batch) is the
critical path; emission keeps it bubble-free. Per pair: dots+exp stream
chunk-by-chunk ("phase A") while the PREVIOUS pair's av matmuls ("phase B",
reading fully-exp'd P) interleave between chunks with no exp gating, so the
tensor queue never head-of-line-blocks the next dots.

Self-contained: hardcodes shapes/sharding; builds + caches one SPMD Bacc graph.
"""

import sys
import numpy as np

if '/opt/trn_rl_repo' not in sys.path:
    sys.path.insert(0, '/opt/trn_rl_repo')
_a = sys.modules.get('antenv')
if _a is not None and '_ro' in getattr(_a, '__file__', ''):
    # purge the read-only copy so antenv resolves to /opt/trn_rl_repo
    for _m in list(sys.modules):
        if _m == 'antenv' or _m.startswith('antenv.'):
            del sys.modules[_m]

import ml_dtypes

EPS = 1e-5
HEADS = 8
DK = 32
DV = 64
B_TOT = 32
N_CORES = 8
B_LOC = B_TOT // N_CORES  # 4 batches per core
C_IN = 256                # input channels
C_V = 512                 # v channels (h*dv)
N = 1024                  # pixels (32*32)
VSTRIDE = DV + 1          # v_aug block: 64 data cols + ones col
N_PAIR = 4 * B_LOC        # 16 head-pairs in the global pipeline

_cache = {}


def _build():
    import concourse.bass as bass
    import concourse.tile as tile
    from concourse import bacc, mybir

    f32 = mybir.dt.float32
    bf16 = mybir.dt.bfloat16
    Exp = mybir.ActivationFunctionType.Exp
    Tanh = mybir.ActivationFunctionType.Tanh
    mult = mybir.AluOpType.mult
    add = mybir.AluOpType.add

    nc = bacc.Bacc("TRN2", target_bir_lowering=False, debug=False,
                   num_devices=N_CORES)

    x_ext = nc.declare_dram_parameter("x", [B_LOC, C_IN, N], bf16, isOutput=False)
    wqT_ext = nc.declare_dram_parameter("wqT", [2, 128, 256], bf16, isOutput=False)
    wkT_ext = nc.declare_dram_parameter("wkT", [2, 128, 256], bf16, isOutput=False)
    wvT_ext = nc.declare_dram_parameter("wvT", [2, 128, 512], bf16, isOutput=False)
    woT_ext = nc.declare_dram_parameter("woT", [4, 128, 256], bf16, isOutput=False)
    shq_ext = nc.declare_dram_parameter("shq", [128, 2], f32, isOutput=False)
    shk_ext = nc.declare_dram_parameter("shk", [128, 2], f32, isOutput=False)
    shv_ext = nc.declare_dram_parameter("shv", [1, 512], bf16, isOutput=False)
    ones_ext = nc.declare_dram_parameter("onesr", [1, 128], bf16, isOutput=False)
    bo_ext = nc.declare_dram_parameter("bo", [128, 2], f32, isOutput=False)
    out_ext = nc.declare_dram_parameter("out", [B_LOC, C_IN, N], bf16, isOutput=True)

    GC = 0.7978845608028654
    GA = GC * 0.044715

    from contextlib import ExitStack
    with tile.TileContext(nc) as tc, ExitStack() as ctx:
        consts = ctx.enter_context(tc.tile_pool(name="consts", bufs=1))
        vpool = ctx.enter_context(tc.tile_pool(name="vaug", bufs=1))
        xbp = ctx.enter_context(tc.tile_pool(name="xb", bufs=2))
        qkp = ctx.enter_context(tc.tile_pool(name="qk", bufs=2))
        pp = ctx.enter_context(tc.tile_pool(name="pp", bufs=2))
        gp = ctx.enter_context(tc.tile_pool(name="gp", bufs=2))
        tmp = ctx.enter_context(tc.tile_pool(name="tmp", bufs=2))
        gbfp = ctx.enter_context(tc.tile_pool(name="gbf", bufs=2))
        rp = ctx.enter_context(tc.tile_pool(name="rp", bufs=4))
        osp = ctx.enter_context(tc.tile_pool(name="os", bufs=4))
        pdp = ctx.enter_context(tc.tile_pool(name="pd", bufs=3, space="PSUM"))
        avp = ctx.enter_context(tc.tile_pool(name="av", bufs=2, space="PSUM"))

        # ---- load constants ----
        wq_sb = [consts.tile([128, 256], bf16, tag=f"wq{t}", name=f"wq{t}") for t in range(2)]
        wk_sb = [consts.tile([128, 256], bf16, tag=f"wk{t}", name=f"wk{t}") for t in range(2)]
        wv_sb = [consts.tile([128, 512], bf16, tag=f"wv{t}", name=f"wv{t}") for t in range(2)]
        wo_sb = [consts.tile([128, 256], bf16, tag=f"wo{t}", name=f"wo{t}") for t in range(4)]
        shq_sb = consts.tile([128, 2], f32, tag="shq", name="shq")
        shk_sb = consts.tile([128, 2], f32, tag="shk", name="shk")
        shv_sb = consts.tile([1, 512], bf16, tag="shv", name="shv")
        ones_sb = consts.tile([1, 128], bf16, tag="ones", name="ones")
        bo_sb = consts.tile([128, 2], f32, tag="bo", name="bo")
        for t in range(2):
            nc.gpsimd.dma_start(out=wq_sb[t][:], in_=wqT_ext.ap()[t])
            nc.gpsimd.dma_start(out=wk_sb[t][:], in_=wkT_ext.ap()[t])
            nc.gpsimd.dma_start(out=wv_sb[t][:], in_=wvT_ext.ap()[t])
        for t in range(4):
            nc.gpsimd.dma_start(out=wo_sb[t][:], in_=woT_ext.ap()[t])
        nc.gpsimd.dma_start(out=shq_sb[:], in_=shq_ext.ap()[:])
        nc.gpsimd.dma_start(out=shk_sb[:], in_=shk_ext.ap()[:])
        nc.gpsimd.dma_start(out=shv_sb[:], in_=shv_ext.ap()[:])
        nc.gpsimd.dma_start(out=ones_sb[:], in_=ones_ext.ap()[:])
        nc.gpsimd.dma_start(out=bo_sb[:], in_=bo_ext.ap()[:])

        # two persistent v_aug buffers (ones columns memset once, data columns
        # rewritten per batch; av reads l from the ones column product)
        vaug = [vpool.tile([128, 64 * VSTRIDE], bf16, tag=f"vaug{i}", name=f"vaug{i}") for i in range(2)]
        nc.vector.memset(vaug[0][:], 1.0)
        nc.vector.memset(vaug[1][:], 1.0)

        # V-projection bias as a precomputed [pixel, channel] plane
        bvs = consts.tile([128, 512], bf16, tag="bvs", name="bvs")
        bps = avp.tile([128, 512], f32, tag="av", name="av")
        nc.tensor.matmul(bps[:], ones_sb[0:1, 0:128], shv_sb[0:1, :],
                         start=True, stop=True)
        nc.vector.tensor_copy(bvs[:], bps[:])

        st = {}   # per-batch tiles: xb, q, k, gbf
        pst = {}  # per-pair tiles: P, g

        def emit_x(b):
            # sync-queue DMA so the x load overlaps the gpsimd-queue weight DMAs
            xb = xbp.tile([128, 2048], bf16, tag="xb", name="xb")
            nc.sync.dma_start(
                out=xb[:].rearrange("p (t n) -> p t n", t=2),
                in_=x_ext.ap()[b].rearrange("(t p) n -> p t n", p=128))
            st[b] = {'xb': xb, 'gbf': []}

        def alloc_qk(b):
            # q2/k2 are partition-rotated (+64) copies so odd dots chunks run
            # on the other two PE row bands -> 4-way row-tile concurrency
            for i, key in ((0, 'q'), (1, 'k'), (2, 'q2'), (3, 'k2')):
                st[b][key] = [qkp.tile([128, N], bf16, tag=f"qk{i}{t}",
                                       name=f"qk{i}{t}") for t in range(2)]

        def emit_qkv_piece(b, piece, piece2):
            """piece 0: Q proj tile piece2, 1: K proj tile piece2,
            2: V j-half piece2."""
            xb = st[b]['xb']
            va = vaug[b % 2]
            # filler pieces use the av pool ([128, 512] halves) so the pd pool
            # stays exclusive to dots and never stalls the exp stream
            if piece in (0, 1):
                w_sb, sh_sb = ((wq_sb, shq_sb), (wk_sb, shk_sb))[piece]
                key = 'q' if piece == 0 else 'k'
                dst = st[b][key]
                dst2 = st[b][key + '2']
                t = piece2
                for ih in range(2):
                    ps = avp.tile([128, 512], f32, tag="av", name="av")
                    for kt in range(2):
                        nc.tensor.matmul(
                            ps[:],
                            w_sb[kt][:, t * 128:(t + 1) * 128],
                            xb[:, kt * 1024 + ih * 512:kt * 1024 + ih * 512 + 512],
                            start=(kt == 0), stop=(kt == 1))
                    nc.vector.tensor_scalar(
                        dst[t][:, ih * 512:ih * 512 + 512], ps[:],
                        sh_sb[:, t:t + 1], None, add)
                # partition-rotated copy (+64) for the odd-chunk dots bands,
                # via SBUF->SBUF DMA to keep it off the DVE
                nc.gpsimd.dma_start(out=dst2[t][0:64, :], in_=dst[t][64:128, :])
                nc.gpsimd.dma_start(out=dst2[t][64:128, :], in_=dst[t][0:64, :])
            else:
                # V projection, transposed ([pixel, channel]); bias added on DVE
                half = piece2
                for cq in range(2):
                    jc = half * 2 + cq
                    ps = avp.tile([128, 512], f32, tag="av", name="av")
                    for kt in range(2):
                        nc.tensor.matmul(
                            ps[:],
                            xb[:, kt * 1024 + jc * 128:kt * 1024 + jc * 128 + 128],
                            wv_sb[kt][:],
                            start=(kt == 0), stop=(kt == 1))
                    src = ps[:].rearrange("p (h d) -> p h d", h=8)
                    bsrc = bvs[:].rearrange("p (h d) -> p h d", h=8)
                    dst = va[:, jc * 8 * VSTRIDE:(jc + 1) * 8 * VSTRIDE].rearrange(
                        "p (h e) -> p h e", h=8)[:, :, 0:DV]
                    nc.vector.tensor_tensor(dst, src, bsrc, add)

        def emit_dots_chunk(q, jc):
            """Phase A: dots for both heads of pair q, chunk jc, plus exp."""
            b, p = q // 4, q % 4
            if jc % 2 == 0:
                q_sb, k_sb = st[b]['q'], st[b]['k']
                sh = 0
            else:
                # odd chunks read the rotated copies and use the other two PE
                # row bands, so chunks jc and jc+1 overlap 4-way in the array
                q_sb, k_sb = st[b]['q2'], st[b]['k2']
                sh = 64
            P = pst[q]['P']
            h0 = 2 * p
            t_q = h0 // 4
            offs = ((32 * (h0 % 4) + sh) % 128, (32 * (h0 % 4) + 32 + sh) % 128)
            pds = [pdp.tile([128, 1024], f32, tag="pd", name="pd")
                   for _ in range(2)]
            # band-alternating issue order -> PE row-tile concurrency
            for ih in range(2):
                for hi in range(2):
                    off = offs[hi]
                    nc.tensor.matmul(
                        pds[hi][:, ih * 512:ih * 512 + 512],
                        k_sb[t_q][off:off + 32, jc * 128:(jc + 1) * 128],
                        q_sb[t_q][off:off + 32, ih * 512:(ih + 1) * 512],
                        start=True, stop=True,
                        tile_position=(off, 0))
            for hi in range(2):
                nc.scalar.activation(
                    P[:, hi * 8192 + jc * 1024:hi * 8192 + (jc + 1) * 1024],
                    pds[hi][:], Exp)

        def emit_av_group(q, gi, from_pd=False):
            """Phase B: one (hi, ih) av accumulation over all 8 chunks of the
            fully-exp'd P of pair q, then its softmax-normalize chain."""
            b, p = q // 4, q % 4
            hi, ih = gi // 2, gi % 2
            va = vaug[b % 2]
            P = pst[q]['P']
            g = pst[q]['g']
            h = 2 * p + hi
            if from_pd:
                # tail only: dots are done, so the pd pool is idle; using it
                # lets all four tail av groups issue without waiting on the
                # avp ring's normalize evictions
                avt = pdp.tile([128, 1024], f32, tag="pd", name="pd")[:, 0:512]
            else:
                avt = avp.tile([128, 512], f32, tag="av", name="av")
            for jc in range(8):
                nc.tensor.matmul(
                    avt[0:65, :],
                    va[:, jc * 8 * VSTRIDE + h * VSTRIDE:
                       jc * 8 * VSTRIDE + h * VSTRIDE + VSTRIDE],
                    P[:, hi * 8192 + jc * 1024 + ih * 512:
                      hi * 8192 + jc * 1024 + ih * 512 + 512],
                    start=(jc == 0), stop=(jc == 7))
            # normalize: copy the l row to partition 0, recip there, broadcast
            # down and multiply into the g tile
            lsb = rp.tile([1, 512], f32, tag="lsb", name="lsb")
            nc.vector.tensor_copy(lsb[0:1, :], avt[64:65, :])
            rh = rp.tile([1, 512], f32, tag="rh", name="rh")
            nc.vector.reciprocal_approx_fast(rh[0:1, :], lsb[0:1, :])
            R = rp.tile([64, 512], f32, tag="R", name="R")
            nc.gpsimd.partition_broadcast(R[:], rh[0:1, :])
            nc.vector.tensor_tensor(
                g[hi * 64:hi * 64 + 64, ih * 512:ih * 512 + 512],
                avt[0:64, :], R[:], mult)

        def emit_gelu(q):
            """tanh-form gelu of pair q's g -> gbf (0.5 folded into wo):
            gelu(x)/0.5 = x * (1 + tanh(GC*x + GA*x^3))"""
            b = q // 4
            g = pst[q]['g']
            u = tmp.tile([128, N], bf16, tag="u", name="u")
            nc.vector.tensor_tensor(u[:], g[:], g[:], mult)
            t2 = tmp.tile([128, N], bf16, tag="t2", name="t2")
            nc.vector.tensor_scalar(t2[:], u[:], GA, GC, mult, add)
            z = tmp.tile([128, N], bf16, tag="z", name="z")
            nc.vector.tensor_tensor(z[:], t2[:], g[:], mult)
            t3 = tmp.tile([128, N], bf16, tag="t3", name="t3")
            nc.scalar.activation(t3[:], z[:], Tanh)
            t4 = tmp.tile([128, N], bf16, tag="t4", name="t4")
            nc.vector.tensor_scalar(t4[:], t3[:], 1.0, None, add)
            gt = gbfp.tile([128, N], bf16, tag=f"gbf{q % 4}", name=f"gbf{q % 4}")
            nc.vector.tensor_tensor(gt[:], t4[:], g[:], mult)
            st[b]['gbf'].append(gt)

        def emit_outconv(b, ot):
            gbf = st[b]['gbf']
            osb = osp.tile([128, N], bf16, tag="osb", name="osb")
            for ih in range(2):
                pso = avp.tile([128, 512], f32, tag="av", name="av")
                for kt in range(4):
                    nc.tensor.matmul(
                        pso[:],
                        wo_sb[kt][:, ot * 128:(ot + 1) * 128],
                        gbf[kt][:, ih * 512:(ih + 1) * 512],
                        start=(kt == 0), stop=(kt == 3))
                nc.vector.tensor_scalar(osb[:, ih * 512:ih * 512 + 512], pso[:],
                                        bo_sb[:, ot:ot + 1], None, add)
            nc.gpsimd.dma_start(
                out=out_ext.ap()[b, ot * 128:(ot + 1) * 128, :], in_=osb[:])

        # warm the ACT table set during the initial DMA wait
        warm = consts.tile([1, 16], bf16, tag="warm", name="warm")
        nc.vector.memset(warm[:], 0.0)
        nc.scalar.activation(warm[:], warm[:], Exp)

        # ---- global pair pipeline ----
        # batch 0 prologue: only t=0 Q/K (needed by pair 0's dots); the rest
        # go to the filler queue (t=1 first used by pair 2, V by pair 1's
        # av-phase of pair 0)
        from collections import deque
        fillq = deque()
        emit_x(0)
        alloc_qk(0)
        emit_qkv_piece(0, 0, 0)
        emit_qkv_piece(0, 1, 0)
        fillq.append((emit_qkv_piece, (0, 0, 1)))
        fillq.append((emit_qkv_piece, (0, 1, 1)))
        for p2 in range(4):
            fillq.append((emit_qkv_piece, (0, 2, p2)))

        def drain_filler():
            if fillq:
                f, args = fillq.popleft()
                f(*args)

        for q in range(N_PAIR):
            b, p = q // 4, q % 4
            # queue this pair's share of next-batch prep / prev-batch outconv
            if p == 0 and b + 1 < B_LOC:
                emit_x(b + 1)
                alloc_qk(b + 1)
                fillq.append((emit_qkv_piece, (b + 1, 0, 0)))
                fillq.append((emit_qkv_piece, (b + 1, 0, 1)))
            elif p == 1:
                if b + 1 < B_LOC:
                    fillq.append((emit_qkv_piece, (b + 1, 1, 0)))
                    fillq.append((emit_qkv_piece, (b + 1, 1, 1)))
                if b >= 1:
                    fillq.append((emit_outconv, (b - 1, 0)))
            elif p == 2:
                if b >= 1:
                    fillq.append((emit_outconv, (b - 1, 1)))
                if b + 1 < B_LOC:
                    fillq.append((emit_qkv_piece, (b + 1, 2, 0)))
                    fillq.append((emit_qkv_piece, (b + 1, 2, 1)))
            elif p == 3 and b + 1 < B_LOC:
                fillq.append((emit_qkv_piece, (b + 1, 2, 2)))
                fillq.append((emit_qkv_piece, (b + 1, 2, 3)))

            pst[q] = {
                'P': pp.tile([128, 2 * 8192], bf16, tag="P", name="P"),
                'g': gp.tile([128, N], bf16, tag="g", name="g"),
            }
            # pair 0 runs even chunks first: odd chunks need the q2/k2 rotated
            # DMA copies, which land while the first even-chunk exps run
            jc_order = (0, 2, 4, 1, 6, 3, 5, 7) if q == 0 else range(8)
            for si, jc in enumerate(jc_order):
                emit_dots_chunk(q, jc)
                # slot 7 stays empty so nothing sits between the last dots of
                # this pair and the first dots of the next on the tensor queue
                if q >= 1:
                    if si < 4:
                        emit_av_group(q - 1, si)
                    elif si == 4:
                        emit_gelu(q - 1)
                        drain_filler()
                    elif si < 7:
                        drain_filler()
                elif si < 7:
                    drain_filler()
        # tail: last pair's phase B + gelu + final outconv
        for gi in range(4):
            emit_av_group(N_PAIR - 1, gi, from_pd=(gi >= 2))
        emit_gelu(N_PAIR - 1)
        while fillq:
            drain_filler()
        emit_outconv(B_LOC - 1, 0)
        emit_outconv(B_LOC - 1, 1)

    nc.compile()
    return nc


def _get_nc():
    if 'nc' not in _cache:
        _cache['nc'] = _build()
    return _cache['nc']


def _fold_weights(inputs):
    """Fold BatchNorms (+ attention scale) into conv weights, host-side."""
    f8 = {k: np.asarray(v, np.float64) for k, v in inputs.items()}
    scale = DK ** -0.5

    def fold(w, g, b, m, v, extra=1.0):
        inv = g / np.sqrt(v + EPS)
        return w * inv[:, None] * extra, (b - m * inv) * extra

    wq_e, shq = fold(f8['wq'], f8['gq'], f8['bq'], f8['mq'], f8['vq'], scale)
    wk_e, shk = fold(f8['wk'], f8['gk'], f8['bk'], f8['mk'], f8['vk'])
    wv_e, shv = fold(f8['wv'], f8['gv'], f8['bv'], f8['mv'], f8['vv'])
    inv_o = f8['go'] / np.sqrt(f8['vo'] + EPS)
    # the 0.5 of the tanh-form gelu is folded in here
    wo_e = f8['wo'] * inv_o[:, None] * 0.5
    bo = inv_o * f8['b_out'] + (f8['be_o'] - f8['mo'] * inv_o)

    bf = ml_dtypes.bfloat16
    return {
        'wqT': np.ascontiguousarray(wq_e.T).reshape(2, 128, 256).astype(bf),
        'wkT': np.ascontiguousarray(wk_e.T).reshape(2, 128, 256).astype(bf),
        'wvT': np.ascontiguousarray(wv_e.T).reshape(2, 128, 512).astype(bf),
        'woT': np.ascontiguousarray(wo_e.T).reshape(4, 128, 256).astype(bf),
        'shq': np.ascontiguousarray(shq.reshape(2, 128).T).astype(np.float32),
        'shk': np.ascontiguousarray(shk.reshape(2, 128).T).astype(np.float32),
        'shv': shv.reshape(1, 512).astype(bf),
        'onesr': np.ones((1, 128), bf),
        'bo': np.ascontiguousarray(bo.reshape(2, 128).T).astype(np.float32),
    }


def kernel_run(inputs, trace=False, trace_kwargs=None):
    from concourse.bass_utils import run_bass_kernel_spmd
    nc = _get_nc()
    consts = _fold_weights(inputs)
    x = np.asarray(inputs['x'], np.float32).reshape(B_TOT, C_IN, N)
    x = x.astype(ml_dtypes.bfloat16)
    in_maps = []
    for c in range(N_CORES):
        m = dict(consts)
        m['x'] = np.ascontiguousarray(x[c * B_LOC:(c + 1) * B_LOC])
        in_maps.append(m)
    res = run_bass_kernel_spmd(nc, in_maps, core_ids=list(range(N_CORES)),
                               trace=trace, **(trace_kwargs or {}))
    out = np.concatenate([res.results[c]['out'] for c in range(N_CORES)], axis=0)
    return out.reshape(B_TOT, C_IN, 32, 32).astype(np.float32), res


def kernel(**inputs) -> np.ndarray:
    out, _ = kernel_run(inputs, trace=False)
    return out


# revision 30
# speedup vs baseline: 1.1021x; 1.1021x over previous
"""Trainium2 Bass kernel for nn_Attention_24215025615017.

8-head spatial attention block (1x1-conv QKV projections with folded BatchNorm,
transposed-softmax attention, exact GELU, output 1x1 conv with folded BN).
Data-parallel over batch: B=32 sharded as 4 batches on each of 8 NeuronCores.

Pipeline design: the softmax exp on the Scalar engine (~64 ACTs/batch) is the
critical path; emission keeps it bubble-free. Per pair: dots+exp stream
chunk-by-chunk ("phase A") while the PREVIOUS pair's av matmuls ("phase B",
reading fully-exp'd P) interleave between chunks with no exp gating, so the
tensor queue never head-of-line-blocks the next dots.

Self-contained: hardcodes shapes/sharding; builds + caches one SPMD Bacc graph.
"""

import sys
import numpy as np

if '/opt/trn_rl_repo' not in sys.path:
    sys.path.insert(0, '/opt/trn_rl_repo')
_a = sys.modules.get('antenv')
if _a is not None and '_ro' in getattr(_a, '__file__', ''):
    # purge the read-only copy so antenv resolves to /opt/trn_rl_repo
    for _m in list(sys.modules):
        if _m == 'antenv' or _m.startswith('antenv.'):
            del sys.modules[_m]

import ml_dtypes

EPS = 1e-5
HEADS = 8
DK = 32
DV = 64
B_TOT = 32
N_CORES = 8
B_LOC = B_TOT // N_CORES  # 4 batches per core
C_IN = 256                # input channels
C_V = 512                 # v channels (h*dv)
N = 1024                  # pixels (32*32)
VSTRIDE = DV + 1          # v_aug block: 64 data cols + ones col
N_PAIR = 4 * B_LOC        # 16 head-pairs in the global pipeline

_cache = {}


def _build():
    import concourse.bass as bass
    import concourse.tile as tile
    from concourse import bacc, mybir

    f32 = mybir.dt.float32
    bf16 = mybir.dt.bfloat16
    Exp = mybir.ActivationFunctionType.Exp
    Tanh = mybir.ActivationFunctionType.Tanh
    mult = mybir.AluOpType.mult
    add = mybir.AluOpType.add

    nc = bacc.Bacc("TRN2", target_bir_lowering=False, debug=False,
                   num_devices=N_CORES)

    x_ext = nc.declare_dram_parameter("x", [B_LOC, C_IN, N], bf16, isOutput=False)
    wqT_ext = nc.declare_dram_parameter("wqT", [2, 128, 256], bf16, isOutput=False)
    wkT_ext = nc.declare_dram_parameter("wkT", [2, 128, 256], bf16, isOutput=False)
    wvT_ext = nc.declare_dram_parameter("wvT", [2, 128, 512], bf16, isOutput=False)
    woT_ext = nc.declare_dram_parameter("woT", [4, 128, 256], bf16, isOutput=False)
    shq_ext = nc.declare_dram_parameter("shq", [128, 2], f32, isOutput=False)
    shk_ext = nc.declare_dram_parameter("shk", [128, 2], f32, isOutput=False)
    shv_ext = nc.declare_dram_parameter("shv", [1, 512], bf16, isOutput=False)
    ones_ext = nc.declare_dram_parameter("onesr", [1, 128], bf16, isOutput=False)
    bo_ext = nc.declare_dram_parameter("bo", [128, 2], f32, isOutput=False)
    out_ext = nc.declare_dram_parameter("out", [B_LOC, C_IN, N], bf16, isOutput=True)

    GC = 0.7978845608028654
    GA = GC * 0.044715

    from contextlib import ExitStack
    with tile.TileContext(nc) as tc, ExitStack() as ctx:
        consts = ctx.enter_context(tc.tile_pool(name="consts", bufs=1))
        vpool = ctx.enter_context(tc.tile_pool(name="vaug", bufs=1))
        xbp = ctx.enter_context(tc.tile_pool(name="xb", bufs=2))
        qkp = ctx.enter_context(tc.tile_pool(name="qk", bufs=2))
        pp = ctx.enter_context(tc.tile_pool(name="pp", bufs=2))
        gp = ctx.enter_context(tc.tile_pool(name="gp", bufs=2))
        tmp = ctx.enter_context(tc.tile_pool(name="tmp", bufs=2))
        gbfp = ctx.enter_context(tc.tile_pool(name="gbf", bufs=2))
        rp = ctx.enter_context(tc.tile_pool(name="rp", bufs=4))
        osp = ctx.enter_context(tc.tile_pool(name="os", bufs=4))
        pdp = ctx.enter_context(tc.tile_pool(name="pd", bufs=3, space="PSUM"))
        avp = ctx.enter_context(tc.tile_pool(name="av", bufs=2, space="PSUM"))

        # ---- load constants ----
        wq_sb = [consts.tile([128, 256], bf16, tag=f"wq{t}", name=f"wq{t}") for t in range(2)]
        wk_sb = [consts.tile([128, 256], bf16, tag=f"wk{t}", name=f"wk{t}") for t in range(2)]
        wv_sb = [consts.tile([128, 512], bf16, tag=f"wv{t}", name=f"wv{t}") for t in range(2)]
        wo_sb = [consts.tile([128, 256], bf16, tag=f"wo{t}", name=f"wo{t}") for t in range(4)]
        shq_sb = consts.tile([128, 2], f32, tag="shq", name="shq")
        shk_sb = consts.tile([128, 2], f32, tag="shk", name="shk")
        shv_sb = consts.tile([1, 512], bf16, tag="shv", name="shv")
        ones_sb = consts.tile([1, 128], bf16, tag="ones", name="ones")
        bo_sb = consts.tile([128, 2], f32, tag="bo", name="bo")
        for t in range(2):
            nc.gpsimd.dma_start(out=wq_sb[t][:], in_=wqT_ext.ap()[t])
            nc.gpsimd.dma_start(out=wk_sb[t][:], in_=wkT_ext.ap()[t])
            nc.gpsimd.dma_start(out=wv_sb[t][:], in_=wvT_ext.ap()[t])
        for t in range(4):
            nc.gpsimd.dma_start(out=wo_sb[t][:], in_=woT_ext.ap()[t])
        nc.gpsimd.dma_start(out=shq_sb[:], in_=shq_ext.ap()[:])
        nc.gpsimd.dma_start(out=shk_sb[:], in_=shk_ext.ap()[:])
        nc.gpsimd.dma_start(out=shv_sb[:], in_=shv_ext.ap()[:])
        nc.gpsimd.dma_start(out=ones_sb[:], in_=ones_ext.ap()[:])
        nc.gpsimd.dma_start(out=bo_sb[:], in_=bo_ext.ap()[:])

        # two persistent v_aug buffers (ones columns memset once, data columns
        # rewritten per batch; av reads l from the ones column product)
        vaug = [vpool.tile([128, 64 * VSTRIDE], bf16, tag=f"vaug{i}", name=f"vaug{i}") for i in range(2)]
        nc.vector.memset(vaug[0][:], 1.0)
        nc.vector.memset(vaug[1][:], 1.0)

        # V-projection bias as a precomputed [pixel, channel] plane
        bvs = consts.tile([128, 512], bf16, tag="bvs", name="bvs")
        bps = avp.tile([128, 512], f32, tag="av", name="av")
        nc.tensor.matmul(bps[:], ones_sb[0:1, 0:128], shv_sb[0:1, :],
                         start=True, stop=True)
        nc.vector.tensor_copy(bvs[:], bps[:])

        st = {}   # per-batch tiles: xb, q, k, gbf
        pst = {}  # per-pair tiles: P, g

        def emit_x(b):
            # sync-queue DMA so the x load overlaps the gpsimd-queue weight DMAs
            xb = xbp.tile([128, 2048], bf16, tag="xb", name="xb")
            nc.sync.dma_start(
                out=xb[:].rearrange("p (t n) -> p t n", t=2),
                in_=x_ext.ap()[b].rearrange("(t p) n -> p t n", p=128))
            st[b] = {'xb': xb, 'gbf': []}

        def alloc_qk(b):
            # q2/k2 are partition-rotated (+64) copies so odd dots chunks run
            # on the other two PE row bands -> 4-way row-tile concurrency
            for i, key in ((0, 'q'), (1, 'k'), (2, 'q2'), (3, 'k2')):
                st[b][key] = [qkp.tile([128, N], bf16, tag=f"qk{i}{t}",
                                       name=f"qk{i}{t}") for t in range(2)]

        def emit_qkv_piece(b, piece, piece2):
            """piece 0: Q proj tile piece2, 1: K proj tile piece2,
            2: V j-half piece2."""
            xb = st[b]['xb']
            va = vaug[b % 2]
            # filler pieces use the av pool ([128, 512] halves) so the pd pool
            # stays exclusive to dots and never stalls the exp stream
            if piece in (0, 1):
                w_sb, sh_sb = ((wq_sb, shq_sb), (wk_sb, shk_sb))[piece]
                key = 'q' if piece == 0 else 'k'
                dst = st[b][key]
                dst2 = st[b][key + '2']
                t = piece2
                for ih in range(2):
                    ps = avp.tile([128, 512], f32, tag="av", name="av")
                    for kt in range(2):
                        nc.tensor.matmul(
                            ps[:],
                            w_sb[kt][:, t * 128:(t + 1) * 128],
                            xb[:, kt * 1024 + ih * 512:kt * 1024 + ih * 512 + 512],
                            start=(kt == 0), stop=(kt == 1))
                    nc.vector.tensor_scalar(
                        dst[t][:, ih * 512:ih * 512 + 512], ps[:],
                        sh_sb[:, t:t + 1], None, add)
                # partition-rotated copy (+64) for the odd-chunk dots bands,
                # via SBUF->SBUF DMA to keep it off the DVE
                nc.gpsimd.dma_start(out=dst2[t][0:64, :], in_=dst[t][64:128, :])
                nc.gpsimd.dma_start(out=dst2[t][64:128, :], in_=dst[t][0:64, :])
            else:
                # V projection, transposed ([pixel, channel]); bias added on DVE
                half = piece2
                for cq in range(2):
                    jc = half * 2 + cq
                    ps = avp.tile([128, 512], f32, tag="av", name="av")
                    for kt in range(2):
                        nc.tensor.matmul(
                            ps[:],
                            xb[:, kt * 1024 + jc * 128:kt * 1024 + jc * 128 + 128],
                            wv_sb[kt][:],
                            start=(kt == 0), stop=(kt == 1))
                    src = ps[:].rearrange("p (h d) -> p h d", h=8)
                    bsrc = bvs[:].rearrange("p (h d) -> p h d", h=8)
                    dst = va[:, jc * 8 * VSTRIDE:(jc + 1) * 8 * VSTRIDE].rearrange(
                        "p (h e) -> p h e", h=8)[:, :, 0:DV]
                    nc.vector.tensor_tensor(dst, src, bsrc, add)

        def emit_dots_chunk(q, jc):
            """Phase A: dots for both heads of pair q, chunk jc, plus exp."""
            b, p = q // 4, q % 4
            if jc % 2 == 0:
                q_sb, k_sb = st[b]['q'], st[b]['k']
                sh = 0
            else:
                # odd chunks read the rotated copies and use the other two PE
                # row bands, so chunks jc and jc+1 overlap 4-way in the array
                q_sb, k_sb = st[b]['q2'], st[b]['k2']
                sh = 64
            P = pst[q]['P']
            h0 = 2 * p
            t_q = h0 // 4
            offs = ((32 * (h0 % 4) + sh) % 128, (32 * (h0 % 4) + 32 + sh) % 128)
            pds = [pdp.tile([128, 1024], f32, tag="pd", name="pd")
                   for _ in range(2)]
            # band-alternating issue order -> PE row-tile concurrency
            for ih in range(2):
                for hi in range(2):
                    off = offs[hi]
                    nc.tensor.matmul(
                        pds[hi][:, ih * 512:ih * 512 + 512],
                        k_sb[t_q][off:off + 32, jc * 128:(jc + 1) * 128],
                        q_sb[t_q][off:off + 32, ih * 512:(ih + 1) * 512],
                        start=True, stop=True,
                        tile_position=(off, 0))
            for hi in range(2):
                nc.scalar.activation(
                    P[:, hi * 8192 + jc * 1024:hi * 8192 + (jc + 1) * 1024],
                    pds[hi][:], Exp)

        def emit_av_group(q, gi, from_pd=False):
            """Phase B: one (hi, ih) av accumulation over all 8 chunks of the
            fully-exp'd P of pair q, then its softmax-normalize chain."""
            b, p = q // 4, q % 4
            hi, ih = gi // 2, gi % 2
            va = vaug[b % 2]
            P = pst[q]['P']
            g = pst[q]['g']
            h = 2 * p + hi
            if from_pd:
                # tail only: dots are done, so the pd pool is idle; using it
                # lets all four tail av groups issue without waiting on the
                # avp ring's normalize evictions
                avt = pdp.tile([128, 1024], f32, tag="pd", name="pd")[:, 0:512]
            else:
                avt = avp.tile([128, 512], f32, tag="av", name="av")
            for jc in range(8):
                nc.tensor.matmul(
                    avt[0:65, :],
                    va[:, jc * 8 * VSTRIDE + h * VSTRIDE:
                       jc * 8 * VSTRIDE + h * VSTRIDE + VSTRIDE],
                    P[:, hi * 8192 + jc * 1024 + ih * 512:
                      hi * 8192 + jc * 1024 + ih * 512 + 512],
                    start=(jc == 0), stop=(jc == 7))
            # normalize: copy the l row to partition 0, recip there, broadcast
            # down and multiply into the g tile
            lsb = rp.tile([1, 512], f32, tag="lsb", name="lsb")
            nc.vector.tensor_copy(lsb[0:1, :], avt[64:65, :])
            rh = rp.tile([1, 512], f32, tag="rh", name="rh")
            nc.vector.reciprocal_approx_fast(rh[0:1, :], lsb[0:1, :])
            R = rp.tile([64, 512], f32, tag="R", name="R")
            nc.gpsimd.partition_broadcast(R[:], rh[0:1, :])
            nc.vector.tensor_tensor(
                g[hi * 64:hi * 64 + 64, ih * 512:ih * 512 + 512],
                avt[0:64, :], R[:], mult)

        def emit_gelu(q):
            """tanh-form gelu of pair q's g -> gbf (0.5 folded into wo):
            gelu(x)/0.5 = x * (1 + tanh(GC*x + GA*x^3))"""
            b = q // 4
            g = pst[q]['g']
            u = tmp.tile([128, N], bf16, tag="u", name="u")
            nc.vector.tensor_tensor(u[:], g[:], g[:], mult)
            t2 = tmp.tile([128, N], bf16, tag="t2", name="t2")
            nc.vector.tensor_scalar(t2[:], u[:], GA, GC, mult, add)
            z = tmp.tile([128, N], bf16, tag="z", name="z")
            nc.vector.tensor_tensor(z[:], t2[:], g[:], mult)
            t3 = tmp.tile([128, N], bf16, tag="t3", name="t3")
            nc.scalar.activation(t3[:], z[:], Tanh)
            t4 = tmp.tile([128, N], bf16, tag="t4", name="t4")
            nc.vector.tensor_scalar(t4[:], t3[:], 1.0, None, add)
            gt = gbfp.tile([128, N], bf16, tag=f"gbf{q % 4}", name=f"gbf{q % 4}")
            nc.vector.tensor_tensor(gt[:], t4[:], g[:], mult)
            st[b]['gbf'].append(gt)

        def emit_outconv(b, ot):
            gbf = st[b]['gbf']
            osb = osp.tile([128, N], bf16, tag="osb", name="osb")
            for ih in range(2):
                pso = avp.tile([128, 512], f32, tag="av", name="av")
                for kt in range(4):
                    nc.tensor.matmul(
                        pso[:],
                        wo_sb[kt][:, ot * 128:(ot + 1) * 128],
                        gbf[kt][:, ih * 512:(ih + 1) * 512],
                        start=(kt == 0), stop=(kt == 3))
                nc.vector.tensor_scalar(osb[:, ih * 512:ih * 512 + 512], pso[:],
                                        bo_sb[:, ot:ot + 1], None, add)
            nc.gpsimd.dma_start(
                out=out_ext.ap()[b, ot * 128:(ot + 1) * 128, :], in_=osb[:])

        # warm the ACT table set during the initial DMA wait
        warm = consts.tile([1, 16], bf16, tag="warm", name="warm")
        nc.vector.memset(warm[:], 0.0)
        nc.scalar.activation(warm[:], warm[:], Exp)

        # ---- global pair pipeline ----
        # batch 0 prologue: only t=0 Q/K (needed by pair 0's dots); the rest
        # go to the filler queue (t=1 first used by pair 2, V by pair 1's
        # av-phase of pair 0)
        from collections import deque
        fillq = deque()
        emit_x(0)
        alloc_qk(0)
        emit_qkv_piece(0, 0, 0)
        emit_qkv_piece(0, 1, 0)
        fillq.append((emit_qkv_piece, (0, 0, 1)))
        fillq.append((emit_qkv_piece, (0, 1, 1)))
        for p2 in range(4):
            fillq.append((emit_qkv_piece, (0, 2, p2)))

        def drain_filler():
            if fillq:
                f, args = fillq.popleft()
                f(*args)

        # global chunk emission order; pair 0 runs even chunks first (odd
        # chunks need the q2/k2 rotated DMA copies, which land while the
        # first even-chunk exps run)
        chunk_seq = [(0, jc) for jc in (0, 2, 4, 1, 6, 3, 5, 7)]
        for q in range(1, N_PAIR):
            chunk_seq += [(q, jc) for jc in range(8)]

        def emit_dots_global(c):
            if c >= len(chunk_seq):
                return
            q, jc = chunk_seq[c]
            if q not in pst:
                pst[q] = {
                    'P': pp.tile([128, 2 * 8192], bf16, tag="P", name="P"),
                    'g': gp.tile([128, N], bf16, tag="g", name="g"),
                }
            emit_dots_chunk(q, jc)

        # dots run two chunks ahead of everything else on the tensor queue so
        # av bursts and fillers never head-of-line-block the exp stream
        emit_dots_global(0)
        emit_dots_global(1)
        for q in range(N_PAIR):
            b, p = q // 4, q % 4
            # queue this pair's share of next-batch prep / prev-batch outconv
            if p == 0 and b + 1 < B_LOC:
                emit_x(b + 1)
                alloc_qk(b + 1)
                fillq.append((emit_qkv_piece, (b + 1, 0, 0)))
                fillq.append((emit_qkv_piece, (b + 1, 0, 1)))
            elif p == 1:
                if b + 1 < B_LOC:
                    fillq.append((emit_qkv_piece, (b + 1, 1, 0)))
                    fillq.append((emit_qkv_piece, (b + 1, 1, 1)))
                if b >= 1:
                    fillq.append((emit_outconv, (b - 1, 0)))
            elif p == 2:
                if b >= 1:
                    fillq.append((emit_outconv, (b - 1, 1)))
                if b + 1 < B_LOC:
                    fillq.append((emit_qkv_piece, (b + 1, 2, 0)))
                    fillq.append((emit_qkv_piece, (b + 1, 2, 1)))
            elif p == 3 and b + 1 < B_LOC:
                fillq.append((emit_qkv_piece, (b + 1, 2, 2)))
                fillq.append((emit_qkv_piece, (b + 1, 2, 3)))

            for si in range(8):
                emit_dots_global(8 * q + si + 2)
                if q >= 1:
                    if si < 4:
                        emit_av_group(q - 1, si)
                    elif si == 4:
                        emit_gelu(q - 1)
                        drain_filler()
                    elif si < 7:
                        drain_filler()
                elif si < 7:
                    drain_filler()
        # tail: last pair's phase B + gelu + final outconv
        for gi in range(4):
            emit_av_group(N_PAIR - 1, gi, from_pd=(gi >= 2))
        emit_gelu(N_PAIR - 1)
        while fillq:
            drain_filler()
        emit_outconv(B_LOC - 1, 0)
        emit_outconv(B_LOC - 1, 1)

    nc.compile()
    return nc


def _get_nc():
    if 'nc' not in _cache:
        _cache['nc'] = _build()
    return _cache['nc']


def _fold_weights(inputs):
    """Fold BatchNorms (+ attention scale) into conv weights, host-side."""
    f8 = {k: np.asarray(v, np.float64) for k, v in inputs.items()}
    scale = DK ** -0.5

    def fold(w, g, b, m, v, extra=1.0):
        inv = g / np.sqrt(v + EPS)
        return w * inv[:, None] * extra, (b - m * inv) * extra

    wq_e, shq = fold(f8['wq'], f8['gq'], f8['bq'], f8['mq'], f8['vq'], scale)
    wk_e, shk = fold(f8['wk'], f8['gk'], f8['bk'], f8['mk'], f8['vk'])
    wv_e, shv = fold(f8['wv'], f8['gv'], f8['bv'], f8['mv'], f8['vv'])
    inv_o = f8['go'] / np.sqrt(f8['vo'] + EPS)
    # the 0.5 of the tanh-form gelu is folded in here
    wo_e = f8['wo'] * inv_o[:, None] * 0.5
    bo = inv_o * f8['b_out'] + (f8['be_o'] - f8['mo'] * inv_o)

    bf = ml_dtypes.bfloat16
    return {
        'wqT': np.ascontiguousarray(wq_e.T).reshape(2, 128, 256).astype(bf),
        'wkT': np.ascontiguousarray(wk_e.T).reshape(2, 128, 256).astype(bf),
        'wvT': np.ascontiguousarray(wv_e.T).reshape(2, 128, 512).astype(bf),
        'woT': np.ascontiguousarray(wo_e.T).reshape(4, 128, 256).astype(bf),
        'shq': np.ascontiguousarray(shq.reshape(2, 128).T).astype(np.float32),
        'shk': np.ascontiguousarray(shk.reshape(2, 128).T).astype(np.float32),
        'shv': shv.reshape(1, 512).astype(bf),
        'onesr': np.ones((1, 128), bf),
        'bo': np.ascontiguousarray(bo.reshape(2, 128).T).astype(np.float32),
    }


def kernel_run(inputs, trace=False, trace_kwargs=None):
    from concourse.bass_utils import run_bass_kernel_spmd
    nc = _get_nc()
    consts = _fold_weights(inputs)
    x = np.asarray(inputs['x'], np.float32).reshape(B_TOT, C_IN, N)
    x = x.astype(ml_dtypes.bfloat16)
    in_maps = []
    for c in range(N_CORES):
        m = dict(consts)
        m['x'] = np.ascontiguousarray(x[c * B_LOC:(c + 1) * B_LOC])
        in_maps.append(m)
    res = run_bass_kernel_spmd(nc, in_maps, core_ids=list(range(N_CORES)),
                               trace=trace, **(trace_kwargs or {}))
    out = np.concatenate([res.results[c]['out'] for c in range(N_CORES)], axis=0)
    return out.reshape(B_TOT, C_IN, 32, 32).astype(np.float32), res


def kernel(**inputs) -> np.ndarray:
    out, _ = kernel_run(inputs, trace=False)
    return out


# revision 31
# speedup vs baseline: 1.1482x; 1.0419x over previous
"""Trainium2 Bass kernel for nn_Attention_24215025615017.

8-head spatial attention block (1x1-conv QKV projections with folded BatchNorm,
transposed-softmax attention, exact GELU, output 1x1 conv with folded BN).
Data-parallel over batch: B=32 sharded as 4 batches on each of 8 NeuronCores.

Pipeline design: the softmax exp on the Scalar engine (~64 ACTs/batch) is the
critical path; emission keeps it bubble-free. Per pair: dots+exp stream
chunk-by-chunk ("phase A") while the PREVIOUS pair's av matmuls ("phase B",
reading fully-exp'd P) interleave between chunks with no exp gating, so the
tensor queue never head-of-line-blocks the next dots.

Self-contained: hardcodes shapes/sharding; builds + caches one SPMD Bacc graph.
"""

import sys
import numpy as np

if '/opt/trn_rl_repo' not in sys.path:
    sys.path.insert(0, '/opt/trn_rl_repo')
_a = sys.modules.get('antenv')
if _a is not None and '_ro' in getattr(_a, '__file__', ''):
    # purge the read-only copy so antenv resolves to /opt/trn_rl_repo
    for _m in list(sys.modules):
        if _m == 'antenv' or _m.startswith('antenv.'):
            del sys.modules[_m]

import ml_dtypes

EPS = 1e-5
HEADS = 8
DK = 32
DV = 64
B_TOT = 32
N_CORES = 8
B_LOC = B_TOT // N_CORES  # 4 batches per core
C_IN = 256                # input channels
C_V = 512                 # v channels (h*dv)
N = 1024                  # pixels (32*32)
VSTRIDE = DV + 1          # v_aug block: 64 data cols + ones col
N_PAIR = 4 * B_LOC        # 16 head-pairs in the global pipeline

_cache = {}


def _build():
    import concourse.bass as bass
    import concourse.tile as tile
    from concourse import bacc, mybir

    f32 = mybir.dt.float32
    bf16 = mybir.dt.bfloat16
    Exp = mybir.ActivationFunctionType.Exp
    Tanh = mybir.ActivationFunctionType.Tanh
    mult = mybir.AluOpType.mult
    add = mybir.AluOpType.add

    nc = bacc.Bacc("TRN2", target_bir_lowering=False, debug=False,
                   num_devices=N_CORES)

    x_ext = nc.declare_dram_parameter("x", [B_LOC, C_IN, N], bf16, isOutput=False)
    wqT_ext = nc.declare_dram_parameter("wqT", [2, 128, 256], bf16, isOutput=False)
    wkT_ext = nc.declare_dram_parameter("wkT", [2, 128, 256], bf16, isOutput=False)
    wvT_ext = nc.declare_dram_parameter("wvT", [2, 128, 512], bf16, isOutput=False)
    woT_ext = nc.declare_dram_parameter("woT", [4, 128, 256], bf16, isOutput=False)
    shq_ext = nc.declare_dram_parameter("shq", [128, 2], f32, isOutput=False)
    shk_ext = nc.declare_dram_parameter("shk", [128, 2], f32, isOutput=False)
    shv_ext = nc.declare_dram_parameter("shv", [1, 512], bf16, isOutput=False)
    ones_ext = nc.declare_dram_parameter("onesr", [1, 128], bf16, isOutput=False)
    bo_ext = nc.declare_dram_parameter("bo", [128, 2], f32, isOutput=False)
    out_ext = nc.declare_dram_parameter("out", [B_LOC, C_IN, N], bf16, isOutput=True)

    GC = 0.7978845608028654
    GA = GC * 0.044715

    from contextlib import ExitStack
    with tile.TileContext(nc) as tc, ExitStack() as ctx:
        consts = ctx.enter_context(tc.tile_pool(name="consts", bufs=1))
        vpool = ctx.enter_context(tc.tile_pool(name="vaug", bufs=1))
        xbp = ctx.enter_context(tc.tile_pool(name="xb", bufs=2))
        qkp = ctx.enter_context(tc.tile_pool(name="qk", bufs=2))
        pp = ctx.enter_context(tc.tile_pool(name="pp", bufs=2))
        gp = ctx.enter_context(tc.tile_pool(name="gp", bufs=2))
        tmp = ctx.enter_context(tc.tile_pool(name="tmp", bufs=2))
        gbfp = ctx.enter_context(tc.tile_pool(name="gbf", bufs=2))
        rp = ctx.enter_context(tc.tile_pool(name="rp", bufs=4))
        osp = ctx.enter_context(tc.tile_pool(name="os", bufs=4))
        pdp = ctx.enter_context(tc.tile_pool(name="pd", bufs=3, space="PSUM"))
        avp = ctx.enter_context(tc.tile_pool(name="av", bufs=2, space="PSUM"))

        # ---- load constants ----
        wq_sb = [consts.tile([128, 256], bf16, tag=f"wq{t}", name=f"wq{t}") for t in range(2)]
        wk_sb = [consts.tile([128, 256], bf16, tag=f"wk{t}", name=f"wk{t}") for t in range(2)]
        wv_sb = [consts.tile([128, 512], bf16, tag=f"wv{t}", name=f"wv{t}") for t in range(2)]
        wo_sb = [consts.tile([128, 256], bf16, tag=f"wo{t}", name=f"wo{t}") for t in range(4)]
        shq_sb = consts.tile([128, 2], f32, tag="shq", name="shq")
        shk_sb = consts.tile([128, 2], f32, tag="shk", name="shk")
        shv_sb = consts.tile([1, 512], bf16, tag="shv", name="shv")
        ones_sb = consts.tile([1, 128], bf16, tag="ones", name="ones")
        bo_sb = consts.tile([128, 2], f32, tag="bo", name="bo")
        for t in range(2):
            nc.gpsimd.dma_start(out=wq_sb[t][:], in_=wqT_ext.ap()[t])
            nc.gpsimd.dma_start(out=wk_sb[t][:], in_=wkT_ext.ap()[t])
            nc.gpsimd.dma_start(out=wv_sb[t][:], in_=wvT_ext.ap()[t])
        for t in range(4):
            nc.gpsimd.dma_start(out=wo_sb[t][:], in_=woT_ext.ap()[t])
        nc.gpsimd.dma_start(out=shq_sb[:], in_=shq_ext.ap()[:])
        nc.gpsimd.dma_start(out=shk_sb[:], in_=shk_ext.ap()[:])
        nc.gpsimd.dma_start(out=shv_sb[:], in_=shv_ext.ap()[:])
        nc.gpsimd.dma_start(out=ones_sb[:], in_=ones_ext.ap()[:])
        nc.gpsimd.dma_start(out=bo_sb[:], in_=bo_ext.ap()[:])

        # two persistent v_aug buffers (ones columns memset once, data columns
        # rewritten per batch; av reads l from the ones column product)
        vaug = [vpool.tile([128, 64 * VSTRIDE], bf16, tag=f"vaug{i}", name=f"vaug{i}") for i in range(2)]
        nc.vector.memset(vaug[0][:], 1.0)
        nc.vector.memset(vaug[1][:], 1.0)

        # V-projection bias as a precomputed [pixel, channel] plane
        bvs = consts.tile([128, 512], bf16, tag="bvs", name="bvs")
        bps = avp.tile([128, 512], f32, tag="av", name="av")
        nc.tensor.matmul(bps[:], ones_sb[0:1, 0:128], shv_sb[0:1, :],
                         start=True, stop=True)
        nc.vector.tensor_copy(bvs[:], bps[:])

        st = {}   # per-batch tiles: xb, q, k, gbf
        pst = {}  # per-pair tiles: P, g

        def emit_x(b):
            # sync-queue DMA so the x load overlaps the gpsimd-queue weight DMAs
            xb = xbp.tile([128, 2048], bf16, tag="xb", name="xb")
            nc.sync.dma_start(
                out=xb[:].rearrange("p (t n) -> p t n", t=2),
                in_=x_ext.ap()[b].rearrange("(t p) n -> p t n", p=128))
            st[b] = {'xb': xb, 'gbf': []}

        def alloc_qk(b):
            # q2/k2 are partition-rotated (+64) copies so odd dots chunks run
            # on the other two PE row bands -> 4-way row-tile concurrency
            for i, key in ((0, 'q'), (1, 'k'), (2, 'q2'), (3, 'k2')):
                st[b][key] = [qkp.tile([128, N], bf16, tag=f"qk{i}{t}",
                                       name=f"qk{i}{t}") for t in range(2)]

        def emit_qkv_piece(b, piece, piece2):
            """piece 0: Q proj tile piece2, 1: K proj tile piece2,
            2: V j-half piece2."""
            xb = st[b]['xb']
            va = vaug[b % 2]
            # filler pieces use the av pool ([128, 512] halves) so the pd pool
            # stays exclusive to dots and never stalls the exp stream
            if piece in (0, 1):
                w_sb, sh_sb = ((wq_sb, shq_sb), (wk_sb, shk_sb))[piece]
                key = 'q' if piece == 0 else 'k'
                dst = st[b][key]
                dst2 = st[b][key + '2']
                t = piece2
                for ih in range(2):
                    ps = avp.tile([128, 512], f32, tag="av", name="av")
                    for kt in range(2):
                        nc.tensor.matmul(
                            ps[:],
                            w_sb[kt][:, t * 128:(t + 1) * 128],
                            xb[:, kt * 1024 + ih * 512:kt * 1024 + ih * 512 + 512],
                            start=(kt == 0), stop=(kt == 1))
                    nc.vector.tensor_scalar(
                        dst[t][:, ih * 512:ih * 512 + 512], ps[:],
                        sh_sb[:, t:t + 1], None, add)
                # partition-rotated copy (+64) for the odd-chunk dots bands,
                # via SBUF->SBUF DMA to keep it off the DVE
                nc.gpsimd.dma_start(out=dst2[t][0:64, :], in_=dst[t][64:128, :])
                nc.gpsimd.dma_start(out=dst2[t][64:128, :], in_=dst[t][0:64, :])
            else:
                # V projection, transposed ([pixel, channel]); bias added on DVE
                half = piece2
                for cq in range(2):
                    jc = half * 2 + cq
                    ps = avp.tile([128, 512], f32, tag="av", name="av")
                    for kt in range(2):
                        nc.tensor.matmul(
                            ps[:],
                            xb[:, kt * 1024 + jc * 128:kt * 1024 + jc * 128 + 128],
                            wv_sb[kt][:],
                            start=(kt == 0), stop=(kt == 1))
                    src = ps[:].rearrange("p (h d) -> p h d", h=8)
                    bsrc = bvs[:].rearrange("p (h d) -> p h d", h=8)
                    dst = va[:, jc * 8 * VSTRIDE:(jc + 1) * 8 * VSTRIDE].rearrange(
                        "p (h e) -> p h e", h=8)[:, :, 0:DV]
                    nc.vector.tensor_tensor(dst, src, bsrc, add)

        def emit_dots_chunk(q, jc):
            """Phase A: dots for both heads of pair q, chunk jc, plus exp."""
            b, p = q // 4, q % 4
            if jc % 2 == 0:
                q_sb, k_sb = st[b]['q'], st[b]['k']
                sh = 0
            else:
                # odd chunks read the rotated copies and use the other two PE
                # row bands, so chunks jc and jc+1 overlap 4-way in the array
                q_sb, k_sb = st[b]['q2'], st[b]['k2']
                sh = 64
            P = pst[q]['P']
            h0 = 2 * p
            t_q = h0 // 4
            offs = ((32 * (h0 % 4) + sh) % 128, (32 * (h0 % 4) + 32 + sh) % 128)
            pds = [pdp.tile([128, 1024], f32, tag="pd", name="pd")
                   for _ in range(2)]
            # band-alternating issue order -> PE row-tile concurrency
            for ih in range(2):
                for hi in range(2):
                    off = offs[hi]
                    nc.tensor.matmul(
                        pds[hi][:, ih * 512:ih * 512 + 512],
                        k_sb[t_q][off:off + 32, jc * 128:(jc + 1) * 128],
                        q_sb[t_q][off:off + 32, ih * 512:(ih + 1) * 512],
                        start=True, stop=True,
                        tile_position=(off, 0))
            for hi in range(2):
                nc.scalar.activation(
                    P[:, hi * 8192 + jc * 1024:hi * 8192 + (jc + 1) * 1024],
                    pds[hi][:], Exp)

        def emit_av_mms(q, gi, avt, jc_lo, jc_hi):
            b, p = q // 4, q % 4
            hi, ih = gi // 2, gi % 2
            va = vaug[b % 2]
            P = pst[q]['P']
            h = 2 * p + hi
            for jc in range(jc_lo, jc_hi):
                nc.tensor.matmul(
                    avt[0:65, :],
                    va[:, jc * 8 * VSTRIDE + h * VSTRIDE:
                       jc * 8 * VSTRIDE + h * VSTRIDE + VSTRIDE],
                    P[:, hi * 8192 + jc * 1024 + ih * 512:
                      hi * 8192 + jc * 1024 + ih * 512 + 512],
                    start=(jc == 0), stop=(jc == 7))

        def emit_av_group(q, gi, from_pd=False):
            """Phase B: one (hi, ih) av accumulation over all 8 chunks of the
            fully-exp'd P of pair q, then its softmax-normalize chain."""
            g = pst[q]['g']
            hi, ih = gi // 2, gi % 2
            if from_pd:
                # tail only: dots are done, so the pd pool is idle; using it
                # lets all four tail av groups issue without waiting on the
                # avp ring's normalize evictions
                avt = pdp.tile([128, 1024], f32, tag="pd", name="pd")[:, 0:512]
            else:
                avt = avp.tile([128, 512], f32, tag="av", name="av")
            emit_av_mms(q, gi, avt, 0, 8)
            emit_av_norm(q, gi, avt)

        def emit_av_norm(q, gi, avt):
            # normalize: copy the l row to partition 0, recip there, broadcast
            # down and multiply into the g tile
            g = pst[q]['g']
            hi, ih = gi // 2, gi % 2
            lsb = rp.tile([1, 512], f32, tag="lsb", name="lsb")
            nc.vector.tensor_copy(lsb[0:1, :], avt[64:65, :])
            rh = rp.tile([1, 512], f32, tag="rh", name="rh")
            nc.vector.reciprocal_approx_fast(rh[0:1, :], lsb[0:1, :])
            R = rp.tile([64, 512], f32, tag="R", name="R")
            nc.gpsimd.partition_broadcast(R[:], rh[0:1, :])
            nc.vector.tensor_tensor(
                g[hi * 64:hi * 64 + 64, ih * 512:ih * 512 + 512],
                avt[0:64, :], R[:], mult)

        def emit_gelu(q):
            """tanh-form gelu of pair q's g -> gbf (0.5 folded into wo):
            gelu(x)/0.5 = x * (1 + tanh(GC*x + GA*x^3))"""
            b = q // 4
            g = pst[q]['g']
            u = tmp.tile([128, N], bf16, tag="u", name="u")
            nc.vector.tensor_tensor(u[:], g[:], g[:], mult)
            t2 = tmp.tile([128, N], bf16, tag="t2", name="t2")
            nc.vector.tensor_scalar(t2[:], u[:], GA, GC, mult, add)
            z = tmp.tile([128, N], bf16, tag="z", name="z")
            nc.vector.tensor_tensor(z[:], t2[:], g[:], mult)
            t3 = tmp.tile([128, N], bf16, tag="t3", name="t3")
            nc.scalar.activation(t3[:], z[:], Tanh)
            t4 = tmp.tile([128, N], bf16, tag="t4", name="t4")
            nc.vector.tensor_scalar(t4[:], t3[:], 1.0, None, add)
            gt = gbfp.tile([128, N], bf16, tag=f"gbf{q % 4}", name=f"gbf{q % 4}")
            nc.vector.tensor_tensor(gt[:], t4[:], g[:], mult)
            st[b]['gbf'].append(gt)

        def emit_outconv(b, ot):
            gbf = st[b]['gbf']
            osb = osp.tile([128, N], bf16, tag="osb", name="osb")
            for ih in range(2):
                pso = avp.tile([128, 512], f32, tag="av", name="av")
                for kt in range(4):
                    nc.tensor.matmul(
                        pso[:],
                        wo_sb[kt][:, ot * 128:(ot + 1) * 128],
                        gbf[kt][:, ih * 512:(ih + 1) * 512],
                        start=(kt == 0), stop=(kt == 3))
                nc.vector.tensor_scalar(osb[:, ih * 512:ih * 512 + 512], pso[:],
                                        bo_sb[:, ot:ot + 1], None, add)
            nc.gpsimd.dma_start(
                out=out_ext.ap()[b, ot * 128:(ot + 1) * 128, :], in_=osb[:])

        # warm the ACT table set during the initial DMA wait
        warm = consts.tile([1, 16], bf16, tag="warm", name="warm")
        nc.vector.memset(warm[:], 0.0)
        nc.scalar.activation(warm[:], warm[:], Exp)

        # ---- global pair pipeline ----
        # batch 0 prologue: only t=0 Q/K (needed by pair 0's dots); the rest
        # go to the filler queue (t=1 first used by pair 2, V by pair 1's
        # av-phase of pair 0)
        from collections import deque
        fillq = deque()
        emit_x(0)
        alloc_qk(0)
        emit_qkv_piece(0, 0, 0)
        emit_qkv_piece(0, 1, 0)
        fillq.append((emit_qkv_piece, (0, 0, 1)))
        fillq.append((emit_qkv_piece, (0, 1, 1)))
        for p2 in range(4):
            fillq.append((emit_qkv_piece, (0, 2, p2)))

        def drain_filler():
            if fillq:
                f, args = fillq.popleft()
                f(*args)

        # global chunk emission order; pair 0 runs even chunks first (odd
        # chunks need the q2/k2 rotated DMA copies, which land while the
        # first even-chunk exps run)
        chunk_seq = [(0, jc) for jc in (0, 2, 4, 1, 6, 3, 5, 7)]
        for q in range(1, N_PAIR):
            chunk_seq += [(q, jc) for jc in range(8)]

        def emit_dots_global(c):
            if c >= len(chunk_seq):
                return
            q, jc = chunk_seq[c]
            if q not in pst:
                pst[q] = {
                    'P': pp.tile([128, 2 * 8192], bf16, tag="P", name="P"),
                    'g': gp.tile([128, N], bf16, tag="g", name="g"),
                }
            emit_dots_chunk(q, jc)

        tail_av = {}
        # dots run two chunks ahead of everything else on the tensor queue so
        # av bursts and fillers never head-of-line-block the exp stream
        emit_dots_global(0)
        emit_dots_global(1)
        for q in range(N_PAIR):
            b, p = q // 4, q % 4
            # queue this pair's share of next-batch prep / prev-batch outconv
            if p == 0 and b + 1 < B_LOC:
                emit_x(b + 1)
                alloc_qk(b + 1)
                fillq.append((emit_qkv_piece, (b + 1, 0, 0)))
                fillq.append((emit_qkv_piece, (b + 1, 0, 1)))
            elif p == 1:
                if b + 1 < B_LOC:
                    fillq.append((emit_qkv_piece, (b + 1, 1, 0)))
                    fillq.append((emit_qkv_piece, (b + 1, 1, 1)))
                if b >= 1:
                    fillq.append((emit_outconv, (b - 1, 0)))
            elif p == 2:
                if b >= 1:
                    fillq.append((emit_outconv, (b - 1, 1)))
                if b + 1 < B_LOC:
                    fillq.append((emit_qkv_piece, (b + 1, 2, 0)))
                    fillq.append((emit_qkv_piece, (b + 1, 2, 1)))
            elif p == 3 and b + 1 < B_LOC:
                fillq.append((emit_qkv_piece, (b + 1, 2, 2)))
                fillq.append((emit_qkv_piece, (b + 1, 2, 3)))

            for si in range(8):
                emit_dots_global(8 * q + si + 2)
                if q >= 1:
                    if si < 4:
                        emit_av_group(q - 1, si)
                    elif si == 4:
                        emit_gelu(q - 1)
                        drain_filler()
                    elif si < 7:
                        if q == N_PAIR - 1 and si == 5:
                            tail_av[0] = avp.tile([128, 512], f32, tag="av",
                                                  name="av")
                            emit_av_mms(q, 0, tail_av[0], 0, 5)
                        elif q == N_PAIR - 1 and si == 6:
                            tail_av[1] = avp.tile([128, 512], f32, tag="av",
                                                  name="av")
                            emit_av_mms(q, 1, tail_av[1], 0, 6)
                            emit_av_mms(q, 0, tail_av[0], 5, 6)
                        else:
                            drain_filler()
                elif si < 7:
                    drain_filler()
        # tail: finish the lagged hi0 groups, then hi1 via the idle pd pool
        qL = N_PAIR - 1
        emit_av_mms(qL, 0, tail_av[0], 6, 8)
        emit_av_mms(qL, 1, tail_av[1], 6, 8)
        emit_av_norm(qL, 0, tail_av[0])
        emit_av_norm(qL, 1, tail_av[1])
        for gi in (2, 3):
            emit_av_group(qL, gi, from_pd=True)
        emit_gelu(N_PAIR - 1)
        while fillq:
            drain_filler()
        emit_outconv(B_LOC - 1, 0)
        emit_outconv(B_LOC - 1, 1)

    nc.compile()
    return nc


def _get_nc():
    if 'nc' not in _cache:
        _cache['nc'] = _build()
    return _cache['nc']


def _fold_weights(inputs):
    """Fold BatchNorms (+ attention scale) into conv weights, host-side."""
    f8 = {k: np.asarray(v, np.float64) for k, v in inputs.items()}
    scale = DK ** -0.5

    def fold(w, g, b, m, v, extra=1.0):
        inv = g / np.sqrt(v + EPS)
        return w * inv[:, None] * extra, (b - m * inv) * extra

    wq_e, shq = fold(f8['wq'], f8['gq'], f8['bq'], f8['mq'], f8['vq'], scale)
    wk_e, shk = fold(f8['wk'], f8['gk'], f8['bk'], f8['mk'], f8['vk'])
    wv_e, shv = fold(f8['wv'], f8['gv'], f8['bv'], f8['mv'], f8['vv'])
    inv_o = f8['go'] / np.sqrt(f8['vo'] + EPS)
    # the 0.5 of the tanh-form gelu is folded in here
    wo_e = f8['wo'] * inv_o[:, None] * 0.5
    bo = inv_o * f8['b_out'] + (f8['be_o'] - f8['mo'] * inv_o)

    bf = ml_dtypes.bfloat16
    return {
        'wqT': np.ascontiguousarray(wq_e.T).reshape(2, 128, 256).astype(bf),
        'wkT': np.ascontiguousarray(wk_e.T).reshape(2, 128, 256).astype(bf),
        'wvT': np.ascontiguousarray(wv_e.T).reshape(2, 128, 512).astype(bf),
        'woT': np.ascontiguousarray(wo_e.T).reshape(4, 128, 256).astype(bf),
        'shq': np.ascontiguousarray(shq.reshape(2, 128).T).astype(np.float32),
        'shk': np.ascontiguousarray(shk.reshape(2, 128).T).astype(np.float32),
        'shv': shv.reshape(1, 512).astype(bf),
        'onesr': np.ones((1, 128), bf),
        'bo': np.ascontiguousarray(bo.reshape(2, 128).T).astype(np.float32),
    }


def kernel_run(inputs, trace=False, trace_kwargs=None):
    from concourse.bass_utils import run_bass_kernel_spmd
    nc = _get_nc()
    consts = _fold_weights(inputs)
    x = np.asarray(inputs['x'], np.float32).reshape(B_TOT, C_IN, N)
    x = x.astype(ml_dtypes.bfloat16)
    in_maps = []
    for c in range(N_CORES):
        m = dict(consts)
        m['x'] = np.ascontiguousarray(x[c * B_LOC:(c + 1) * B_LOC])
        in_maps.append(m)
    res = run_bass_kernel_spmd(nc, in_maps, core_ids=list(range(N_CORES)),
                               trace=trace, **(trace_kwargs or {}))
    out = np.concatenate([res.results[c]['out'] for c in range(N_CORES)], axis=0)
    return out.reshape(B_TOT, C_IN, 32, 32).astype(np.float32), res


def kernel(**inputs) -> np.ndarray:
    out, _ = kernel_run(inputs, trace=False)
    return out
